# revision 11
# baseline (speedup 1.0000x reference)
"""Multi-head attention (B=2, S=2048, D=1024, H=16) on 8 NeuronCores.

Sharding: batch x head-group (2 batches x 4 groups of 4 heads).

Single software-pipelined loop, paced by the ScalarE exp stream (the hard
floor: 16.8M exps/core at 128 lanes = ~110us). Per kc-pair iteration
(one head-pair block, 256 keys):
  - 4 fp16 score matmuls (K=64), row-tiled so the two heads of the pair
    run concurrently on PE row groups 0:64 / 64:128
  - 2 exp ACTIVATEs (fp32 PSUM -> fp16 SBUF)
  - 4 fp16 attn@V matmuls with the per-head row-sums fused as a 1/4 ones
    column appended to V (M=65)
Q/K projections run in fp8 DoubleRow (2x PE throughput; their
quantization noise averages out in the softmax) with weights scaled x16
host-side to clear fp8e4 subnormals (compensated in the exp scale).
V / output projections stay fp16: value-path quantization propagates
directly to the output.  All projections + output projection + softmax
normalization are background work interleaved into the loop's PE slack
via a 2-buffer PSUM ring.
PSUM: scores 2x[128,1024]x2bufs (4 banks) + outA/outB (2) + bg ring (2).
Host: y[b] = sum_g y_g^T.T / 4 + bv @ Wo.T + bo.  K-bias drops out of
softmax; Q-bias applied on device; V-bias commutes through the attention
average and is folded host-side.
"""
import numpy as np

B = 2
S = 2048
D = 1024
H = 16
DK = 64
G = 4              # head-groups (cores per batch)
HG = H // G        # heads per group = 4
DH = HG * DK       # group dims = 256
NQB = S // 512     # query blocks
NKC = S // 128     # key chunks
NKP = NKC // 2     # key-chunk pairs per block = 8
KCD = D // 128     # d_model chunks
VCW = 260          # v cols per key chunk: 4 x [v(64)|1]
LAG = 8            # attn@V trails scores by LAG kc-pair iterations

_CACHE = {}


def _build_nc():
    import concourse.tile as tile
    import concourse.bacc as bacc
    from concourse import mybir
    from contextlib import ExitStack

    F32 = mybir.dt.float32
    F16 = mybir.dt.float16
    F8 = mybir.dt.float8e4
    DR = mybir.MatmulPerfMode.DoubleRow
    Exp = mybir.ActivationFunctionType.Exp

    nc = bacc.Bacc("TRN2", target_bir_lowering=False, debug=False)

    xq_d = nc.dram_tensor("xq", [D, S], F16, kind="ExternalInput").ap()
    xk_d = nc.dram_tensor("xk", [D, S], F16, kind="ExternalInput").ap()
    xv_d = nc.dram_tensor("xv", [D, S], F16, kind="ExternalInput").ap()
    wq_d = nc.dram_tensor("wq", [KCD, 128, DH], F16, kind="ExternalInput").ap()
    wk_d = nc.dram_tensor("wk", [KCD, 128, DH], F16, kind="ExternalInput").ap()
    wv_d = nc.dram_tensor("wv", [KCD, 128, DH], F16, kind="ExternalInput").ap()
    wo_d = nc.dram_tensor("wo", [2, 128, D], F16, kind="ExternalInput").ap()
    bq_d = nc.dram_tensor("bq", [128, 2], F32, kind="ExternalInput").ap()
    sel_d = nc.dram_tensor("sel", [128, 64], F16, kind="ExternalInput").ap()
    y_d = nc.dram_tensor("y", [D, S], F16, kind="ExternalOutput").ap()

    with tile.TileContext(nc) as tc, ExitStack() as ctx:
        sbw = ctx.enter_context(tc.tile_pool(name="sbw", bufs=1))
        sbx = ctx.enter_context(tc.tile_pool(name="sbx", bufs=1))
        sbd = ctx.enter_context(tc.tile_pool(name="sbd", bufs=1))
        sbe = ctx.enter_context(tc.tile_pool(name="sbe", bufs=1))
        sbo = ctx.enter_context(tc.tile_pool(name="sbo", bufs=1))
        ps = ctx.enter_context(tc.tile_pool(name="ps", bufs=1, space="PSUM"))

        # ---- static SBUF tiles ------------------------------------------
        wk_t = sbw.tile([128, KCD * DH], F16)
        wq_t = sbw.tile([128, KCD * DH], F16)
        wv_t = sbw.tile([128, KCD * DH], F16)
        wo_t = sbw.tile([128, 2 * D], F16)
        bq_t = sbw.tile([128, 2], F32)
        sel_t = sbw.tile([128, 64], F16)
        recip = sbw.tile([128, 1024], F16)

        xk_t = sbx.tile([128, KCD * S], F16)
        xq_t = sbx.tile([128, KCD * S], F16)
        xv_t = sbx.tile([128, KCD * S], F16)

        kt_t = [sbd.tile([128, S], F16, name=f"kt{p}") for p in range(2)]
        qt_t = [sbd.tile([128, S], F16, name=f"qt{p}") for p in range(2)]
        v_all = sbd.tile([128, NKC * VCW], F16)
        outsc = sbd.tile([128, 2 * S], F16)

        nc.gpsimd.memset(recip[:], 0.0)
        v4 = v_all[:].rearrange("p (t h f) -> p t h f", t=NKC, h=HG)
        for tb in range(NKC):
            nc.gpsimd.memset(v4[:, tb, :, 64:65], 0.25)

        # ---- DMAs: deadline-ordered; K-side on sync queue, Q/V on the
        # gpsimd queue so the two streams load in parallel ----------------
        nc.sync.dma_start(
            wk_t[:].rearrange("p (c f) -> p c f", c=KCD),
            wk_d.transpose([1, 0, 2]))
        nc.sync.dma_start(bq_t[:], bq_d)
        for kc in range(KCD):   # xk keys 0:512
            nc.sync.dma_start(xk_t[:, kc * S:kc * S + 512],
                              xk_d[kc * 128:(kc + 1) * 128, 0:512])
        nc.gpsimd.dma_start(
            wq_t[:].rearrange("p (c f) -> p c f", c=KCD),
            wq_d.transpose([1, 0, 2]))
        for kc in range(KCD):   # xq queries 0:512
            nc.gpsimd.dma_start(xq_t[:, kc * S:kc * S + 512],
                                xq_d[kc * 128:(kc + 1) * 128, 0:512])
        for kc in range(KCD):   # xk keys 512:2048
            nc.sync.dma_start(xk_t[:, kc * S + 512:(kc + 1) * S],
                              xk_d[kc * 128:(kc + 1) * 128, 512:S])
        nc.gpsimd.dma_start(
            wv_t[:].rearrange("p (c f) -> p c f", c=KCD),
            wv_d.transpose([1, 0, 2]))
        for kc in range(KCD):   # xv keys 0:1024
            nc.gpsimd.dma_start(xv_t[:, kc * S:kc * S + 1024],
                                xv_d[kc * 128:(kc + 1) * 128, 0:1024])
        nc.sync.dma_start(sel_t[:], sel_d)
        nc.sync.dma_start(
            wo_t[:].rearrange("p (c f) -> p c f", c=2),
            wo_d.transpose([1, 0, 2]))
        for kc in range(KCD):   # xq queries 512:1024
            nc.gpsimd.dma_start(xq_t[:, kc * S + 512:kc * S + 1024],
                                xq_d[kc * 128:(kc + 1) * 128, 512:1024])
        for kc in range(KCD):   # xv keys 1024:2048
            nc.gpsimd.dma_start(xv_t[:, kc * S + 1024:(kc + 1) * S],
                                xv_d[kc * 128:(kc + 1) * 128, 1024:S])
        for kc in range(KCD):   # xq queries 1024:2048
            nc.gpsimd.dma_start(xq_t[:, kc * S + 1024:(kc + 1) * S],
                                xq_d[kc * 128:(kc + 1) * 128, 1024:S])

        # ---- background projection work (bg PSUM ring) ------------------
        def bg_tile():
            return ps.tile([128, 512], F32, name="bg", tag="bg", bufs=2)

        def kacc(pb, kb):
            a = bg_tile()
            for kc in range(KCD):
                nc.tensor.matmul(
                    a[:], wk_t[:, kc * DH + pb * 128:kc * DH + (pb + 1) * 128],
                    xk_t[:, kc * S + kb * 512:kc * S + (kb + 1) * 512],
                    start=(kc == 0), stop=(kc == KCD - 1))
            with nc.allow_low_precision(reason="fp16 scores"):
                nc.vector.tensor_copy(kt_t[pb][:, kb * 512:(kb + 1) * 512],
                                      a[:])

        def qacc(pb, qb):
            a = bg_tile()
            for kc in range(KCD):
                nc.tensor.matmul(
                    a[:], wq_t[:, kc * DH + pb * 128:kc * DH + (pb + 1) * 128],
                    xq_t[:, kc * S + qb * 512:kc * S + (qb + 1) * 512],
                    start=(kc == 0), stop=(kc == KCD - 1))
            with nc.allow_low_precision(reason="fp16 scores"):
                nc.vector.tensor_scalar_add(
                    qt_t[pb][:, qb * 512:(qb + 1) * 512], a[:],
                    bq_t[:, pb:pb + 1])

        def vacc(tb):
            a = bg_tile()
            for kc in range(KCD):
                nc.tensor.matmul(
                    a[:, 0:DH], xv_t[:, kc * S + tb * 128:kc * S + (tb + 1) * 128],
                    wv_t[:, kc * DH:(kc + 1) * DH],
                    start=(kc == 0), stop=(kc == KCD - 1))
            src = a[:, 0:DH].rearrange("p (h f) -> p h f", h=HG)
            with nc.allow_low_precision(reason="fp16 attn v"):
                nc.vector.tensor_copy(v4[:, tb, :, 0:64], src)

        # background work schedule: deadlines — kt(0,kb) by it 2kb, v
        # chunks (2j, 2j+1) by it j+LAG, qt(0,qb) by it 8qb, pair-1
        # k/q by it 32+
        bg_sched = {
            1: [("v", 0), ("v", 1), ("k", 0, 1)],
            2: [("v", 2), ("v", 3)],
            3: [("v", 4), ("v", 5), ("k", 0, 2)],
            4: [("v", 6), ("v", 7)],
            5: [("v", 8), ("v", 9), ("k", 0, 3)],
            6: [("v", 10), ("v", 11), ("q", 0, 1)],
            7: [("v", 12), ("v", 13)],
            8: [("v", 14), ("v", 15)],
            10: [("q", 0, 2)],
            14: [("q", 0, 3)],
            18: [("k", 1, 0)],
            20: [("k", 1, 1)],
            22: [("k", 1, 2)],
            24: [("k", 1, 3)],
            26: [("q", 1, 0)],
            30: [("q", 1, 1)],
            34: [("q", 1, 2)],
            38: [("q", 1, 3)],
        }

        def run_bg(item):
            if item[0] == "k":
                kacc(item[1], item[2])
            elif item[0] == "q":
                qacc(item[1], item[2])
            else:
                vacc(item[1])

        # ---- attention-loop pieces --------------------------------------
        blocks = [(pair * NQB + qb, pair, qb)
                  for pair in range(2) for qb in range(NQB)]
        ets = {}
        outs = {}

        def scores(it):
            bi = it // NKP
            _, pair, qb = blocks[bi]
            kp = it % NKP
            ktp, qtp = kt_t[pair], qt_t[pair]
            scA = ps.tile([128, 1024], F32, name="scA", tag="sc", bufs=2)
            scB = ps.tile([128, 1024], F32, name="scB", tag="sc", bufs=2)
            qsl = slice(qb * 512, (qb + 1) * 512)
            for j in range(2):  # j = key chunk within pair
                ksl = slice((2 * kp + j) * 128, (2 * kp + j + 1) * 128)
                nc.tensor.matmul(scA[:, j * 512:(j + 1) * 512],
                                 ktp[0:64, ksl], qtp[0:64, qsl],
                                 start=True, stop=True)
                nc.tensor.matmul(scB[:, j * 512:(j + 1) * 512],
                                 ktp[64:128, ksl], qtp[64:128, qsl],
                                 start=True, stop=True)
            etA = sbe.tile([128, 1024], F16, name="etA", tag="et",
                           bufs=2 * LAG + 2)
            etB = sbe.tile([128, 1024], F16, name="etB", tag="et",
                           bufs=2 * LAG + 2)
            ets[it] = (etA, etB)
            with nc.allow_low_precision(reason="fp16 attn weights"):
                nc.scalar.activation(etA[:], scA[:], Exp, scale=0.125)
                nc.scalar.activation(etB[:], scB[:], Exp, scale=0.125)

        def attn_v(jt):
            bj = jt // NKP
            jp = jt % NKP
            _, pair, qb = blocks[bj]
            if jp == 0:
                outs[bj] = (
                    ps.tile([128, 512], F32, name="outA", tag="oA", bufs=1),
                    ps.tile([128, 512], F32, name="outB", tag="oB", bufs=1))
            outA, outB = outs[bj]
            etA, etB = ets.pop(jt)
            for h, (o, et) in enumerate([(outA, etA), (outB, etB)]):
                for j in range(2):
                    vap = v4[:, 2 * jp + j, pair * 2 + h, 0:65]
                    nc.tensor.matmul(o[0:65, :], vap,
                                     et[:, j * 512:(j + 1) * 512],
                                     start=(jp == 0 and j == 0),
                                     stop=(jp == NKP - 1 and j == 1))

        def norm(bj):
            _, pair, qb = blocks[bj]
            outA, outB = outs.pop(bj)
            with nc.allow_low_precision(reason="fp16 rowsum"):
                nc.vector.tensor_copy(recip[64:65, 0:512], outA[64:65, :])
                nc.vector.tensor_copy(recip[64:65, 512:1024], outB[64:65, :])
            bcA = bg_tile()
            nc.tensor.matmul(bcA[0:64, :], sel_t[:], recip[:, 0:512],
                             start=True, stop=True)
            bcB = bg_tile()
            nc.tensor.matmul(bcB[0:64, :], sel_t[:], recip[:, 512:1024],
                             start=True, stop=True)
            bc_sb = sbo.tile([64, 1024], F32, name="bc_sb", tag="bcr", bufs=2)
            nc.vector.reciprocal_approx_fast(bc_sb[:, 0:512], bcA[0:64, :])
            nc.vector.reciprocal_approx_fast(bc_sb[:, 512:1024], bcB[0:64, :])
            osl = slice(pair * S + qb * 512, pair * S + (qb + 1) * 512)
            with nc.allow_low_precision(reason="fp16 out"):
                nc.vector.tensor_mul(outsc[0:64, osl], outA[0:64, :],
                                     bc_sb[:, 0:512])
                bB = sbo.tile([64, 512], F16, name="bB", tag="bB", bufs=2)
                nc.vector.tensor_mul(bB[:], outB[0:64, :],
                                     bc_sb[:, 512:1024])
            # partition shift 0:64 -> 64:128 (engines cannot cross partitions)
            nc.gpsimd.dma_start(outsc[64:128, osl], bB[:])

        ysb = {}

        def p3_piece(qb, ypb):
            if ypb == 0:
                ysb[qb] = sbo.tile([128, 8 * 512], F16, name="ysb",
                                   tag="ysb", bufs=1)
            ysb_c = ysb[qb]
            yacc = bg_tile()
            for kc2 in range(2):
                nc.tensor.matmul(
                    yacc[:],
                    wo_t[:, kc2 * D + ypb * 128:kc2 * D + (ypb + 1) * 128],
                    outsc[:, kc2 * S + qb * 512:kc2 * S + (qb + 1) * 512],
                    start=(kc2 == 0), stop=(kc2 == 1))
            with nc.allow_low_precision(reason="fp16 y"):
                nc.vector.tensor_copy(ysb_c[:, ypb * 512:(ypb + 1) * 512],
                                      yacc[:])
            if ypb % 2 == 1:
                nc.sync.dma_start(
                    y_d[(ypb - 1) * 128:(ypb + 1) * 128,
                        qb * 512:(qb + 1) * 512]
                    .rearrange("(c p) f -> p c f", p=128),
                    ysb_c[:, (ypb - 1) * 512:(ypb + 1) * 512]
                    .rearrange("p (c f) -> p c f", c=2))

        # p3(qb) pieces spread one per iteration after norm of block 4+qb
        # (emitted at iteration 8*(4+qb)+7+LAG)
        p3_at = {}
        for qb in range(NQB):
            for j in range(8):
                p3_at[8 * qb + 41 + LAG + j] = (qb, j)

        # ---- preamble: first block's K/Q --------------------------------
        kacc(0, 0)
        qacc(0, 0)

        # ---- main loop --------------------------------------------------
        NIT = len(blocks) * NKP
        for it in range(NIT + LAG):
            if it < NIT:
                for item in bg_sched.get(it, []):
                    run_bg(item)
            jt = it - LAG
            if jt >= 0:
                attn_v(jt)
            if it < NIT:
                scores(it)
            if jt >= 0 and jt % NKP == NKP - 1:
                norm(jt // NKP)
            if it in p3_at:
                p3_piece(*p3_at[it])
        for it in range(NIT + LAG, NIT + 64):
            if it in p3_at:
                p3_piece(*p3_at[it])

    nc.compile()
    return nc


def _get_nc():
    if "nc" not in _CACHE:
        _CACHE["nc"] = _build_nc()
    return _CACHE["nc"]


def kernel(q, k, v, Wq, bq, Wk, bk, Wv, bv, Wo, bo, _trace=False, _tmpdir=None):
    import ml_dtypes
    from concourse.bass_utils import run_bass_kernel_spmd

    F8NP = ml_dtypes.float8_e4m3

    def to8(a):
        return np.clip(np.asarray(a, np.float32), -240, 240).astype(F8NP)

    q = np.asarray(q, np.float32)
    k = np.asarray(k, np.float32)
    v = np.asarray(v, np.float32)
    Wq = np.asarray(Wq, np.float32)
    Wk = np.asarray(Wk, np.float32)
    Wv = np.asarray(Wv, np.float32)
    Wo = np.asarray(Wo, np.float32)
    bq = np.asarray(bq, np.float32)
    bv = np.asarray(bv, np.float32)
    bo = np.asarray(bo, np.float32)

    nc = _get_nc()

    sel = np.zeros((128, 64), np.float16)
    sel[64, :] = 1.0

    xT = {}
    for b in range(B):
        xT[("q", b)] = np.ascontiguousarray(q[b].T).astype(np.float16)
        xT[("k", b)] = np.ascontiguousarray(k[b].T).astype(np.float16)
        xT[("v", b)] = np.ascontiguousarray(v[b].T).astype(np.float16)

    # Q/K weights x16 host-side to clear fp8e4 subnormals; the 256x score
    # scale folds into the exp scale.  K-bias drops out of softmax.
    in_maps = []
    for c in range(8):
        b, g = c // G, c % G
        gr = slice(g * DH, (g + 1) * DH)
        in_maps.append({
            "xq": xT[("q", b)],
            "xk": xT[("k", b)],
            "xv": xT[("v", b)],
            "wq": np.ascontiguousarray(Wq[gr, :].T.astype(np.float16)).reshape(KCD, 128, DH),
            "wk": np.ascontiguousarray(Wk[gr, :].T.astype(np.float16)).reshape(KCD, 128, DH),
            "wv": np.ascontiguousarray(Wv[gr, :].T.astype(np.float16)).reshape(KCD, 128, DH),
            "wo": np.ascontiguousarray(Wo[:, gr].T.astype(np.float16)).reshape(2, 128, D),
            "bq": np.ascontiguousarray(bq[gr].reshape(2, 128).T),
            "sel": sel,
        })

    kwargs = {}
    if _trace:
        kwargs = dict(trace=True, tmpdir=_tmpdir)
    res = run_bass_kernel_spmd(nc, in_maps, core_ids=list(range(8)), **kwargs)

    # outsc = 4 x attnout (1/4 ones column); y = wo @ outsc = 4x partial
    bias_row = bv @ Wo.T + bo                     # [D]
    out = np.empty((B, S, D), np.float32)
    for b in range(B):
        acc = np.zeros((S, D), np.float32)
        for g in range(G):
            acc += res.results[b * G + g]["y"].T.astype(np.float32)
        out[b] = acc / 4.0 + bias_row[None, :]
    if _trace:
        out = (out, res)
    return out
